# revision 24
# baseline (speedup 1.0000x reference)
"""Trainium2 Bass kernel for nn_BlurredBoundaryAdaptor.

out[b,t,d] = c[b,t,d] + silu(w0[d]*c[b,t-1,d] + w1[d]*c[b,t,d] + w2[d]*c[b,t+1,d] + bias[d])
where c = c_text * mask, mask[b,t] = 0 at dropped phone-boundary positions
(boundaries = cumsum(dur), dropped where drop_rand < 0.8).

Sharding: batch row b -> NeuronCore b (8 rows, 8 cores).

v2 layout (bf16 streaming): the [T, D] row is processed transposed as [D, T]
(host pre-transposes and casts to bf16; tolerance is 2e-2, bf16 costs ~5e-3):
  - HBM traffic halves vs fp32 (the kernel is DMA-bound at ~360 GB/s/core),
  - the depthwise conv taps are free-dim slices consumed by diagonal 128x128
    bf16 matmuls on the tensor engine (full rate), accumulating 3 taps in
    fp32 PSUM,
  - silu + per-channel bias is one scalar-engine activation per PSUM tile,
    writing bf16,
  - mask multiply and the residual add run on DVE in the 2x_1P packed mode
    (all operands bf16, unit stride, 4B-aligned via a 4-column front pad),
  - the boundary mask is built on-device each pass: cumsum via
    tensor_tensor_scan, block offsets via a strict-lower-triangular matmul,
    scatter of bf16 zeros via indirect DMA into a DRAM scratch row, then
    gpsimd partition_broadcast; the mask pool is double-buffered so pass k+1's
    scatter chain overlaps pass k's main loop (the fp32 baseline serialized
    ~60us of mask build into every pass).
"""

import sys

for _p in ("/opt/trn_rl_repo", "/opt/pypackages"):
    if _p not in sys.path:
        sys.path.insert(0, _p)

import numpy as np

B, T, D, N = 8, 8192, 512, 512
NCORES = 8
DROP_P = 0.8

DG = D // 128           # 4 d-groups of 128 channels
TCH = 4096              # t-chunk (free dim) per tile
NTC = T // TCH          # t-chunks
SUB = 512               # matmul free dim (one PSUM bank)
PS = 2048               # psum tile free dim (4 banks; 2 tiles fill PSUM)
PAD = 4                 # front pad so DVE bf16 slices stay 4B-aligned
SCRATCH = 16384         # DRAM mask scratch elements (>= 2*T)

_CACHE = {}


def _emit_mask_scatter(nc, bass, mybir, mpool, ppool, io):
    """Scatter phase of the mask build: boundary indices -> bf16 {0,1} row in
    DRAM scratch -> read back as mask_row [1, T] in SBUF.

    Emitted one pass AHEAD of the main loop that consumes it, so the whole
    chain (DVE scan -> PE offsets -> gpsimd scatters -> DRAM readback)
    overlaps the previous pass's tile work instead of serializing at the
    pass boundary.
    """
    f32 = mybir.dt.float32
    bf16 = mybir.dt.bfloat16
    i32 = mybir.dt.int32
    Alu = mybir.AluOpType
    dur, drop_rand, mask_dram, ltri_sb = io
    KB = N // 128  # boundaries per partition (4)

    # indices in partition-major [128, KB] layout (boundary n=KB*p+k at [p,k]):
    # per-partition prefix scan of the KB-element blocks + strict-lower-
    # triangular matmul for the cross-partition block offsets.
    dur_i = mpool.tile([128, KB], i32, tag="dur_i")
    drop_sb = mpool.tile([128, KB], f32, tag="drop")
    nc.sync.dma_start(out=dur_i[:], in_=dur.rearrange("(p k) -> p k", k=KB))
    nc.sync.dma_start(out=drop_sb[:], in_=drop_rand.rearrange("(p k) -> p k", k=KB))
    dur_f = mpool.tile([128, KB], f32, tag="dur_f")
    nc.vector.tensor_copy(dur_f[:], dur_i[:])
    scan = mpool.tile([128, KB], f32, tag="scan")
    nc.vector.tensor_tensor_scan(
        out=scan[:], data0=dur_f[:], data1=dur_f[:], initial=0.0,
        op0=Alu.add, op1=Alu.bypass)
    offs_ps = ppool.tile([128, 1], f32, tag="conv")
    nc.tensor.matmul(out=offs_ps[:], lhsT=ltri_sb[:], rhs=scan[:, KB - 1:KB],
                     start=True, stop=True)
    offs = mpool.tile([128, 1], f32, tag="offs_sb")
    nc.vector.tensor_copy(offs[:], offs_ps[:])
    bounds = mpool.tile([128, KB], f32, tag="bounds")
    nc.vector.tensor_scalar(
        out=bounds[:], in0=scan[:], scalar1=offs[:, 0:1], scalar2=None,
        op0=Alu.add)
    # keep = (drop_rand >= DROP_P) -> 1.0 ; idx = bounds + keep*T (kept
    # boundaries land in the unused [T, 2T) half of the scratch row)
    keep = mpool.tile([128, KB], f32, tag="keep")
    nc.vector.tensor_scalar(
        out=keep[:], in0=drop_sb[:], scalar1=float(DROP_P), scalar2=None,
        op0=Alu.is_ge)
    idx_f = mpool.tile([128, KB], f32, tag="idx_f")
    nc.vector.scalar_tensor_tensor(
        out=idx_f[:], in0=keep[:], scalar=float(T), in1=bounds[:],
        op0=Alu.mult, op1=Alu.add)
    idx_pm = mpool.tile([128, KB], i32, tag="idx_pm")
    nc.vector.tensor_copy(idx_pm[:], idx_f[:])

    # ones -> mask_dram, then scatter bf16 zeros at idx
    ones_sb = mpool.tile([128, SCRATCH // 128], bf16, tag="ones")
    nc.vector.memset(ones_sb[:], 1.0)
    nc.sync.dma_start(
        out=mask_dram.rearrange("(p f) one -> p (f one)", p=128),
        in_=ones_sb[:])
    zeros_sb = mpool.tile([128, 1], bf16, tag="zeros")
    nc.vector.memset(zeros_sb[:], 0.0)
    for j in range(KB):
        # out AP sliced to [128, 1]: the offsets are flat element indices into
        # the scratch row (not bounds-checked against the AP extent), and the
        # smaller AP keeps SWDGE descriptor generation at 128 descriptors.
        nc.gpsimd.indirect_dma_start(
            out=mask_dram[0:128, :],
            out_offset=bass.IndirectOffsetOnAxis(ap=idx_pm[:, j:j + 1], axis=0),
            in_=zeros_sb[:, :],
            in_offset=None)
    # load the mask row back to partition 0
    mask_row = mpool.tile([1, T], bf16, tag="mask_row")
    nc.sync.dma_start(out=mask_row[:], in_=mask_dram[0:T, 0][None, :])
    return mask_row


def _emit_mask_broadcast(nc, mybir, mpool, ppool, ones_lhsT, mask_row):
    """Broadcast mask_row to all 128 partitions via PE (ones[1,128]^T @ row)
    into PSUM, then scalar-engine copies to SBUF bf16.

    Deliberately NOT gpsimd partition_broadcast: DVE and GPSIMD share an
    exclusive SBUF port pair, and DVE is ~80% busy with mask-mult + residual;
    PE and the scalar engine have their own SBUF ports and headroom.
    Emitted mid-pass (after a few tiles of the previous pass) so the PE
    queue reaches these matmuls only after the scatter chain has finished.
    """
    f32 = mybir.dt.float32
    bf16 = mybir.dt.bfloat16
    mask_bcast = mpool.tile([128, T + 2 * PAD], bf16, tag="mask_bcast")
    nc.vector.memset(mask_bcast[:, 0:PAD], 0.0)
    nc.vector.memset(mask_bcast[:, T + PAD:T + 2 * PAD], 0.0)
    for ch in range(T // PS):
        psum_t = ppool.tile([128, PS], f32, tag="conv")
        for s in range(PS // SUB):
            c0 = ch * PS + s * SUB
            nc.tensor.matmul(
                out=psum_t[:, s * SUB:(s + 1) * SUB],
                lhsT=ones_lhsT[0:1, :],
                rhs=mask_row[0:1, c0:c0 + SUB],
                start=True, stop=True)
        nc.scalar.activation(
            out=mask_bcast[:, PAD + ch * PS:PAD + (ch + 1) * PS],
            in_=psum_t[:], func=mybir.ActivationFunctionType.Copy,
            bias=0.0, scale=1.0)
    return mask_bcast


def _emit_tile_front(nc, mybir, pools, io, act, mask_bcast, ci, g):
    """Load + mask-multiply + conv matmuls + silu for one [128, TCH] tile.

    x_t/c_t tile column j <-> t = t0 - PAD + j. Data (with conv halo) lives in
    cols PAD-1 .. TCH+PAD; the residual slice [PAD, TCH+PAD) and the mask
    multiply slices start at even element offsets so DVE runs bf16
    tensor_tensor in the packed 2x mode. Split per PS-half so the first
    matmul only waits for half a mask-multiply.
    """
    f32 = mybir.dt.float32
    bf16 = mybir.dt.bfloat16
    Alu = mybir.AluOpType
    mpool, iopool, wpool, ppool = pools
    xT, outT, w_sb, bias_sb = io
    act_func = (mybir.ActivationFunctionType.Silu if act == "silu"
                else mybir.ActivationFunctionType.Sigmoid)
    W = TCH + 2 * PAD
    t0 = ci * TCH

    x_t = iopool.tile([128, W], bf16, tag="x")
    # data cols [PAD-1, TCH+PAD] <-> t in [t0-1, t0+TCH]
    if ci == 0:
        nc.vector.memset(x_t[:, PAD - 1:PAD], 0.0)
        nc.sync.dma_start(
            out=x_t[:, PAD:TCH + PAD + 1],
            in_=xT[g * 128:(g + 1) * 128, 0:TCH + 1])
    elif ci == NTC - 1:
        nc.vector.memset(x_t[:, TCH + PAD:TCH + PAD + 1], 0.0)
        nc.sync.dma_start(
            out=x_t[:, PAD - 1:TCH + PAD],
            in_=xT[g * 128:(g + 1) * 128, t0 - 1:T])
    else:
        nc.sync.dma_start(
            out=x_t[:, PAD - 1:TCH + PAD + 1],
            in_=xT[g * 128:(g + 1) * 128, t0 - 1:t0 + TCH + 1])

    c_t = wpool.tile([128, W], bf16, tag="c")
    out_t = iopool.tile([128, TCH], bf16, tag="out")
    for h in range(TCH // PS):
        # c = x * mask on DVE (bf16 2x mode; segment h covers the c_t cols
        # its taps need: [2, PS+PAD+2) then on to [.., TCH+PAD+2))
        lo = 2 if h == 0 else h * PS + PAD + 2
        hi = (h + 1) * PS + PAD + 2 if h < TCH // PS - 1 else TCH + PAD + 2
        nc.vector.tensor_tensor(
            out=c_t[:, lo:hi],
            in0=x_t[:, lo:hi],
            in1=mask_bcast[:, t0 + lo:t0 + hi],
            op=Alu.mult)

        # conv taps as diagonal bf16 matmuls accumulated in fp32 PSUM;
        # output col u (t = t0+u) reads c_t cols u+PAD-1 .. u+PAD+1
        psum_t = ppool.tile([128, PS], f32, tag="conv")
        for s in range(PS // SUB):
            base = h * PS + s * SUB
            for tap in range(3):
                nc.tensor.matmul(
                    out=psum_t[:, s * SUB:(s + 1) * SUB],
                    lhsT=w_sb[:, (g * 3 + tap) * 128:(g * 3 + tap + 1) * 128],
                    rhs=c_t[:, base + PAD - 1 + tap:base + PAD - 1 + tap + SUB],
                    start=(tap == 0), stop=(tap == 2))
        # silu(conv + bias) on the scalar engine, straight into out_t
        nc.scalar.activation(
            out=out_t[:, h * PS:(h + 1) * PS], in_=psum_t[:],
            func=act_func,
            bias=bias_sb[:, g:g + 1], scale=1.0)
    return (c_t, out_t, g, t0)


def _emit_tile_back(nc, mybir, io, front):
    """Residual add + store for a tile emitted SKEW tiles earlier.

    The skew keeps DVE's in-order queue flowing: by the time DVE reaches
    this residual, the tile's silu finished long ago, so DVE never idles
    waiting on the PE->Act chain (which would also stall every later
    mask-multiply behind it in the queue)."""
    bf16 = mybir.dt.bfloat16
    Alu = mybir.AluOpType
    xT, outT, w_sb, bias_sb = io
    c_t, out_t, g, t0 = front
    for h in range(TCH // PS):
        # residual add on DVE (bf16 2x mode; 4B-aligned slices)
        nc.vector.tensor_tensor(
            out=out_t[:, h * PS:(h + 1) * PS],
            in0=out_t[:, h * PS:(h + 1) * PS],
            in1=c_t[:, h * PS + PAD:(h + 1) * PS + PAD],
            op=Alu.add)
        nc.sync.dma_start(
            out=outT[g * 128:(g + 1) * 128, t0 + h * PS:t0 + (h + 1) * PS],
            in_=out_t[:, h * PS:(h + 1) * PS])


def _build_program(act="silu", passes=1):
    import concourse.bacc as bacc
    import concourse.tile as tile
    import concourse.mybir as mybir
    from concourse import bass

    f32 = mybir.dt.float32
    bf16 = mybir.dt.bfloat16
    i32 = mybir.dt.int32

    nc = bacc.Bacc("TRN2", target_bir_lowering=False, debug=False,
                   enable_asserts=True, num_devices=1)

    xT = nc.dram_tensor("xT", [D, T], bf16, kind="ExternalInput").ap()
    dur = nc.dram_tensor("dur", [N], i32, kind="ExternalInput").ap()
    drop_rand = nc.dram_tensor("drop_rand", [N], f32, kind="ExternalInput").ap()
    wdiag = nc.dram_tensor("wdiag", [DG * 3, 128, 128], bf16,
                           kind="ExternalInput").ap()
    bias_pg = nc.dram_tensor("bias_pg", [128, DG], f32, kind="ExternalInput").ap()
    outT = nc.dram_tensor("outT", [D, T], bf16, kind="ExternalOutput").ap()
    ltri = nc.dram_tensor("ltri", [128, 128], f32, kind="ExternalInput").ap()
    # Two scratch rows for the scattered mask, alternating per pass so pass
    # k+1's ones-fill never waits on pass k's readback.
    mask_drams = [
        nc.dram_tensor(f"mask_scratch{i}", [SCRATCH, 1], bf16, kind="Internal").ap()
        for i in range(2)
    ]

    with tile.TileContext(nc) as tc:
        with (
            tc.tile_pool(name="const", bufs=1) as cpool,
            tc.tile_pool(name="mask", bufs=2) as mpool,
            tc.tile_pool(name="io", bufs=4) as iopool,
            tc.tile_pool(name="work", bufs=4) as wpool,
            tc.tile_pool(name="psum", bufs=2, space="PSUM") as ppool,
        ):
            # ---- constants ----
            w_sb = cpool.tile([128, DG * 3 * 128], bf16, tag="weights")
            for i in range(DG * 3):
                nc.sync.dma_start(out=w_sb[:, i * 128:(i + 1) * 128], in_=wdiag[i])
            bias_sb = cpool.tile([128, DG], f32, tag="bias")
            nc.sync.dma_start(out=bias_sb[:], in_=bias_pg[:, :])
            ltri_sb = cpool.tile([128, 128], f32, tag="ltri")
            nc.sync.dma_start(out=ltri_sb[:], in_=ltri[:, :])
            ones_lhsT = cpool.tile([1, 128], bf16, tag="ones_lhsT")
            nc.vector.memset(ones_lhsT[:], 1.0)

            pools = (mpool, iopool, wpool, ppool)
            main_io = (xT, outT, w_sb, bias_sb)

            def scatter(k):
                return _emit_mask_scatter(
                    nc, bass, mybir, mpool, ppool,
                    (dur, drop_rand, mask_drams[k % 2], ltri_sb))

            def bcast(row):
                return _emit_mask_broadcast(
                    nc, mybir, mpool, ppool, ones_lhsT, row)

            # Software-pipelined on two levels:
            #  - pass k+1's mask scatter chain is emitted before pass k's
            #    tiles (runs during them); its broadcast mid-pass k;
            #  - each tile's residual+store trails its front half by SKEW
            #    tiles, so DVE's in-order queue never parks on a residual
            #    that waits for the PE->Act chain (which would stall every
            #    later mask-multiply queued behind it, draining the whole
            #    pipeline at pass boundaries).
            SKEW = 2
            mask = bcast(scatter(0))
            pend = []
            for k in range(passes):
                row_next = scatter(k + 1) if k + 1 < passes else None
                next_mask = None
                for ci in range(NTC):
                    for g in range(DG):
                        if ci * DG + g == DG and row_next is not None:
                            next_mask = bcast(row_next)
                        pend.append(_emit_tile_front(
                            nc, mybir, pools, main_io, act, mask, ci, g))
                        if len(pend) > SKEW:
                            _emit_tile_back(nc, mybir, main_io, pend.pop(0))
                mask = next_mask
            for front in pend:
                _emit_tile_back(nc, mybir, main_io, front)

    nc.compile()
    return nc


def _get_nc(act="silu", passes=1):
    key = (act, passes)
    if key not in _CACHE:
        _CACHE[key] = _build_program(act, passes)
    return _CACHE[key]


def _bf16():
    import concourse.mybir as mybir
    return mybir.dt.np(mybir.dt.bfloat16)


def _host_prep(c_text, dur, drop_rand, blur_w, blur_b):
    """Per-core input maps. Weights -> per-group diagonal lhsT matrices."""
    bf = _bf16()
    xT = np.ascontiguousarray(
        np.asarray(c_text, dtype=np.float32).transpose(0, 2, 1)).astype(bf)  # [B,D,T]
    w = np.asarray(blur_w, dtype=np.float32).reshape(D, 3)
    wd = np.zeros((DG * 3, 128, 128), dtype=np.float32)
    for g in range(DG):
        for tap in range(3):
            np.fill_diagonal(wd[g * 3 + tap], w[g * 128:(g + 1) * 128, tap])
    wd = wd.astype(bf)
    bias_pg = np.ascontiguousarray(
        np.asarray(blur_b, dtype=np.float32).reshape(DG, 128).T)  # [128, DG]
    # ltri[q, p] = 1 iff q < p  (lhsT for the block-offset matmul)
    ltri = np.triu(np.ones((128, 128), np.float32), k=1)
    in_maps = []
    for b in range(B):
        in_maps.append({
            "xT": xT[b],
            "dur": np.ascontiguousarray(dur[b]).astype(np.int32),
            "drop_rand": np.ascontiguousarray(drop_rand[b]).astype(np.float32),
            "wdiag": wd,
            "bias_pg": bias_pg,
            "ltri": ltri,
        })
    return in_maps


def kernel(c_text, dur, drop_rand, blur_w, blur_b):
    from concourse.bass_utils import run_bass_kernel_spmd

    nc = _get_nc()
    in_maps = _host_prep(c_text, dur, drop_rand, blur_w, blur_b)
    r = run_bass_kernel_spmd(nc, in_maps, core_ids=list(range(NCORES)))
    out = np.stack([np.asarray(r.results[b]["outT"]).astype(np.float32)
                    for b in range(B)])  # [B, D, T]
    return np.ascontiguousarray(out.transpose(0, 2, 1))


# ---------------------------------------------------------------------------
# Timing support (used by test.py, not by the grading harness).
# ---------------------------------------------------------------------------

def _make_repeat_fn(nc, in_maps, repeats):
    """Sharded jitted callable that launches the NEFF `repeats` times per
    dispatch (device queue serializes them), so one ~80ms axon round-trip
    amortizes over `repeats` executions. Returns per-exec output sums to
    keep every execution live."""
    import jax
    import jax.numpy as jnp
    import numpy as _np
    import concourse.mybir as mybir
    from jax.sharding import Mesh, PartitionSpec, NamedSharding
    from jax.experimental.shard_map import shard_map
    from concourse.bass2jax import (_bass_exec_p, install_neuronx_cc_hook,
                                    partition_id_tensor)

    install_neuronx_cc_hook()
    n_cores = len(in_maps)
    partition_name = nc.partition_id_tensor.name if nc.partition_id_tensor else None
    in_names, out_names, out_avals, zero_outs = [], [], [], []
    for alloc in nc.m.functions[0].allocations:
        if not isinstance(alloc, mybir.MemoryLocationSet):
            continue
        name = alloc.memorylocations[0].name
        if alloc.kind == "ExternalInput":
            if name != partition_name:
                in_names.append(name)
        elif alloc.kind == "ExternalOutput":
            shape = tuple(alloc.tensor_shape)
            dtype = mybir.dt.np(alloc.dtype)
            out_names.append(name)
            out_avals.append(jax.core.ShapedArray(shape, dtype))
            zero_outs.append(_np.zeros(shape, dtype))
    all_in_names = list(in_names) + list(out_names)
    if partition_name is not None:
        all_in_names.append(partition_name)

    def _body(*args):
        ops = list(args)
        if partition_name is not None:
            ops.append(partition_id_tensor())
        acc = []
        for _ in range(repeats):
            outs = _bass_exec_p.bind(
                *ops,
                out_avals=tuple(out_avals),
                in_names=tuple(all_in_names),
                out_names=tuple(out_names),
                lowering_input_output_aliases=(),
                sim_require_finite=True,
                sim_require_nnan=True,
                nc=nc,
            )
            acc.append(sum(jnp.sum(o.astype(jnp.float32), keepdims=True)[:1]
                           for o in outs))
        return (jnp.concatenate(acc),)

    devices = jax.devices()[:n_cores]
    mesh = Mesh(np.asarray(devices), ("core",))
    n_in = len(in_names) + len(out_names)
    fn = jax.jit(shard_map(_body, mesh=mesh,
                           in_specs=(PartitionSpec("core"),) * n_in,
                           out_specs=(PartitionSpec("core"),),
                           check_rep=False),
                 keep_unused=True)
    sharding = NamedSharding(mesh, PartitionSpec("core"))
    dev_in = [
        jax.device_put(
            np.concatenate(
                [np.asarray(in_maps[c][nm])[None] for c in range(n_cores)],
                axis=0).reshape(n_cores * np.asarray(in_maps[0][nm]).shape[0],
                                *np.asarray(in_maps[0][nm]).shape[1:]),
            sharding)
        for nm in in_names
    ] + [
        jax.device_put(np.zeros((n_cores * z.shape[0], *z.shape[1:]), z.dtype),
                       sharding)
        for z in zero_outs
    ]
    return fn, dev_in


def time_kernel_batched(inputs, passes=16, reps=(1, 9), rounds=7):
    """Steady-state per-pass time via double differencing:

      t_run(P) = (t(K_hi, P) - t(K_lo, P)) / (K_hi - K_lo)   [per-NEFF-run]
      per-pass = (t_run(passes) - t_run(1)) / (passes - 1)

    K executions ride one jitted dispatch (one axon round-trip), so relay
    noise is divided by K_hi-K_lo instead of swamping a single-run delta.
    """
    import time as _t
    import jax

    in_maps = _host_prep(**inputs)
    klo, khi = reps
    fns = {}
    for P in (1, passes):
        nc = _get_nc("silu", P)
        for K in (klo, khi):
            fns[(P, K)] = _make_repeat_fn(nc, in_maps, K)

    # warm up every variant (compile + first dispatch)
    for key in fns:
        fn, dev_in = fns[key]
        jax.block_until_ready(fn(*dev_in))

    def once(key):
        fn, dev_in = fns[key]
        t0 = _t.perf_counter()
        jax.block_until_ready(fn(*dev_in))
        return _t.perf_counter() - t0

    runs = {k: [] for k in fns}
    order = [(1, klo), (1, khi), (passes, klo), (passes, khi)]
    for r in range(rounds):
        for key in (order if r % 2 == 0 else order[::-1]):
            runs[key].append(once(key))

    med = {k: sorted(v)[len(v) // 2] for k, v in runs.items()}
    t_run1 = (med[(1, khi)] - med[(1, klo)]) / (khi - klo)
    t_runP = (med[(passes, khi)] - med[(passes, klo)]) / (khi - klo)
    per_pass = (t_runP - t_run1) / (passes - 1)
    print(f"t_run(1)={t_run1*1e6:.1f}us  t_run({passes})={t_runP*1e6:.1f}us  "
          f"per-pass={per_pass*1e6:.1f}us")
    return per_pass * 1e9


def _make_timed_fn(nc, in_maps, reduce_outputs=True):
    """Sharded jitted callable over 8 cores with device-resident buffers so it
    can be re-dispatched for timing."""
    import jax
    import numpy as _np
    import concourse.mybir as mybir
    from jax.sharding import Mesh, PartitionSpec, NamedSharding
    from jax.experimental.shard_map import shard_map
    from concourse.bass2jax import (_bass_exec_p, install_neuronx_cc_hook,
                                    partition_id_tensor)

    install_neuronx_cc_hook()
    n_cores = len(in_maps)
    partition_name = nc.partition_id_tensor.name if nc.partition_id_tensor else None
    in_names, out_names, out_avals, zero_outs = [], [], [], []
    for alloc in nc.m.functions[0].allocations:
        if not isinstance(alloc, mybir.MemoryLocationSet):
            continue
        name = alloc.memorylocations[0].name
        if alloc.kind == "ExternalInput":
            if name != partition_name:
                in_names.append(name)
        elif alloc.kind == "ExternalOutput":
            shape = tuple(alloc.tensor_shape)
            dtype = mybir.dt.np(alloc.dtype)
            out_names.append(name)
            out_avals.append(jax.core.ShapedArray(shape, dtype))
            zero_outs.append(_np.zeros(shape, dtype))
    n_params = len(in_names)
    all_in_names = list(in_names) + list(out_names)
    if partition_name is not None:
        all_in_names.append(partition_name)

    def _body(*args):
        ops = list(args)
        if partition_name is not None:
            ops.append(partition_id_tensor())
        outs = _bass_exec_p.bind(
            *ops,
            out_avals=tuple(out_avals),
            in_names=tuple(all_in_names),
            out_names=tuple(out_names),
            lowering_input_output_aliases=(),
            sim_require_finite=True,
            sim_require_nnan=True,
            nc=nc,
        )
        return tuple(outs)

    devices = jax.devices()[:n_cores]
    mesh = Mesh(np.asarray(devices), ("core",))
    in_specs = (PartitionSpec("core"),) * (n_params + len(out_names))
    out_specs = (PartitionSpec("core"),) * len(out_names)
    fn = jax.jit(shard_map(_body, mesh=mesh, in_specs=in_specs,
                           out_specs=out_specs, check_rep=False),
                 keep_unused=True)
    concat_in = [
        np.concatenate([np.asarray(in_maps[c][nm])[None] for c in range(n_cores)],
                       axis=0).reshape(n_cores * np.asarray(in_maps[0][nm]).shape[0],
                                       *np.asarray(in_maps[0][nm]).shape[1:])
        for nm in in_names
    ]
    sharding = NamedSharding(mesh, PartitionSpec("core"))
    dev_in = [jax.device_put(a, sharding) for a in concat_in]

    def _dev_zeros():
        # allocate on device (jitted fill) — device_put of host zeros would
        # stream tens of MB through the axon relay and pollute the timing
        # window with background transfers
        import jax.numpy as jnp

        outs = []
        for z in zero_outs:
            shape = (n_cores * z.shape[0], *z.shape[1:])
            mk = jax.jit(lambda s=shape, d=z.dtype: jnp.zeros(s, d),
                         out_shardings=sharding)
            outs.append(mk())
        return outs

    def make_zero_sets(n):
        # output buffers get consumed (aliased into NEFF outputs) per call —
        # pre-stage one set per timing iteration
        sets = [_dev_zeros() for _ in range(n)]
        jax.block_until_ready(sets)
        return sets

    if reduce_outputs:
        # only a [8]-vector of per-shard sums crosses the axon relay: blocking
        # on the raw per-core outputs marshals them to the client (~74ms/call)
        import jax.numpy as jnp
        inner = fn

        def _sums(*args):
            outs = inner(*args)
            return jax.jit(shard_map(
                lambda *os: tuple(
                    jnp.sum(o.astype(jnp.float32), keepdims=True)[:, 0]
                    for o in os),
                mesh=mesh,
                in_specs=(PartitionSpec("core"),) * len(outs),
                out_specs=(PartitionSpec("core"),) * len(outs),
                check_rep=False))(*outs)

        fn = _sums

    return fn, dev_in, make_zero_sets


def _time_pair(fnA, inA, mzA, fnB, inB, mzB, iters):
    """Interleaved A/B timing: returns median per-pair (tB - tA).

    The axon relay's ~75-125ms per-call overhead drifts on minute scales, so
    two sequential measurement blocks don't subtract cleanly — alternate the
    two programs and difference within each pair instead.
    """
    import time as _t
    import jax
    # ABBA quads: cancels both slow drift and first-vs-second-call order bias.
    quads = (iters + 3) // 4
    zA = mzA(2 * quads + 1)
    zB = mzB(2 * quads + 1)
    jax.block_until_ready(fnA(*inA, *zA[0]))
    jax.block_until_ready(fnB(*inB, *zB[0]))

    def run(fn, ins, z):
        t0 = _t.perf_counter()
        jax.block_until_ready(fn(*ins, *z))
        return _t.perf_counter() - t0

    deltas, tAs = [], []
    for q in range(quads):
        a1 = run(fnA, inA, zA[2 * q + 1])
        b1 = run(fnB, inB, zB[2 * q + 1])
        b2 = run(fnB, inB, zB[2 * q + 2])
        a2 = run(fnA, inA, zA[2 * q + 2])
        deltas.append(((b1 + b2) - (a1 + a2)) / 2)
        tAs += [a1, a2]
    deltas.sort()
    print("   quad deltas (us):",
          " ".join(f"{d*1e6:.0f}" for d in deltas))
    med = deltas[len(deltas) // 2]
    tAs.sort()
    return med, tAs[len(tAs) // 2]


def time_kernel(inputs, iters=20, passes=16):
    """Marginal per-pass device time: (t(passes) - t(1)) / (passes - 1).

    Cancels per-dispatch overhead (axon round trip, NEFF launch, transfers).
    """
    in_maps = _host_prep(**inputs)
    nc1 = _get_nc("silu", 1)
    fn1, in1, mz1 = _make_timed_fn(nc1, in_maps)
    ncK = _get_nc("silu", passes)
    fnK, inK, mzK = _make_timed_fn(ncK, in_maps)
    delta, t1 = _time_pair(fn1, in1, mz1, fnK, inK, mzK, iters)
    per_pass = delta / (passes - 1)
    print(f"t(1 pass)~{t1*1e6:.1f}us  median[t({passes})-t(1)]={delta*1e6:.1f}us  "
          f"marginal per-pass={per_pass*1e6:.1f}us")
    return per_pass * 1e9


def time_kernel_robust(inputs, passes=(1, 65), samples=40):
    """Marginal per-pass: (t(P1) - t(P0)) / (P1 - P0), from alternating
    blocking calls with cluster statistics.

    The axon relay adds ~100ms of per-call overhead whose noise is one-sided
    (network spikes) plus rare anomalously-fast dips, so both the mean and
    the raw min are biased; the tight cluster between the 15th and 50th
    percentile is stable call-to-call. A large P1 (65 passes ~ 3.5ms device
    time) gives the differential enough signal to clear the residual jitter.
    """
    import gc
    import time as _t
    import jax

    in_maps = _host_prep(**inputs)
    p0, p1 = passes
    fn0, in0, mz0 = _make_timed_fn(_get_nc("silu", p0), in_maps)
    fn1, in1, mz1 = _make_timed_fn(_get_nc("silu", p1), in_maps)
    z0 = mz0(samples + 1)
    z1 = mz1(samples + 1)
    jax.block_until_ready(fn0(*in0, *z0[0]))
    jax.block_until_ready(fn1(*in1, *z1[0]))

    def run(fn, ins, z):
        t0 = _t.perf_counter()
        jax.block_until_ready(fn(*ins, *z))
        return _t.perf_counter() - t0

    gc_was_enabled = gc.isenabled()
    gc.disable()
    try:
        t0s, t1s = [], []
        for i in range(1, samples + 1):
            if i % 2 == 0:
                t0s.append(run(fn0, in0, z0[i]))
                t1s.append(run(fn1, in1, z1[i]))
            else:
                t1s.append(run(fn1, in1, z1[i]))
                t0s.append(run(fn0, in0, z0[i]))
    finally:
        if gc_was_enabled:
            gc.enable()

    def cluster_med(ts):
        s = sorted(ts)
        lo, hi = int(len(s) * 0.15), max(int(len(s) * 0.5), int(len(s) * 0.15) + 1)
        seg = s[lo:hi]
        return seg[len(seg) // 2]

    est = (cluster_med(t1s) - cluster_med(t0s)) / (p1 - p0)
    # spread across nearby rank choices = confidence indicator
    alts = []
    for frac in (0.2, 0.3, 0.4):
        i0 = int(len(t0s) * frac)
        alts.append((sorted(t1s)[i0] - sorted(t0s)[i0]) / (p1 - p0))
    print(f"per-pass cluster estimate={est*1e6:.1f}us  "
          f"rank-matched alts(us): " + " ".join(f"{a*1e6:.1f}" for a in alts))
    return est * 1e9
